# revision 88
# baseline (speedup 1.0000x reference)
"""Degraded bicycle rollout kernel for Trainium2 (8 NeuronCores, data-parallel on batch).

Restructure (validated vs reference in numpy; measured rel err ~5e-4 vs 2e-2 tol):
  - Serial 80-step speed recurrence replaced by 3 parallel scans using
        relu-prefix(a)_t = A_t - runmin(min(A,0))_t,  A = s0 + cumsum(a)
    (drops the 1e-6 epsilon inside sqrt; error <= ~1e-3 absolute, non-compounding).
  - Per-partition scalars (deg-derived scales, s0, x0 columns) precomputed on host.
  - tanh/sigmoid outputs in fp16 (halves their SBUF footprint).
  - 4 chunks x 8 rollouts, 3 staging buffers so chunk pipelines overlap; work
    balanced DVE/ACT/GPSIMD; psi+beta fused via affine_then_add; psi0 injected
    into the yaw cumsum at segment starts; ax/ay from diffs of 10*v.
  - All phase-A tanh/sigmoid first, then all trig -> exactly 2 activation-table
    loads (enforced with dep helpers; the greedy table loader thrashes otherwise).

Layout per core: 4096 rollouts = 128 partitions x 32; partition p holds
rollouts p*32..p*32+31, all from batch bc = p//2 (per-batch scalars are
per-partition scalars). Free dim rollout-major: f = n*80 + t.
"""

import sys

sys.path.insert(0, "/opt/trn_rl_repo")

import numpy as np

B, L, H = 512, 64, 80
NCORES = 8
BC = B // NCORES          # 64 batches per core
R = BC * L                # 4096 rollouts per core
P = 128
NPT = R // P              # 32 rollouts per partition
F = NPT * H               # 2560 elements per partition
HP1 = H + 1
CW = 12                   # output channels
SW = HP1 * CW             # 972 staging words per rollout
NCH = 4                   # chunks
NG = NPT // NCH           # 8 rollouts per partition per chunk
CF = NG * H               # 640 free elems per chunk
CHW = NG * SW             # 7776 staging words per partition per chunk
QW = NG * H * 3           # 1920 ctrl words per partition per quarter
DT = 0.1
WB = 2.8
PI = float(np.pi)
BIG = 1e30

_BUILT = None


def _build_kernel():
    import concourse.bass as bass
    import concourse.bacc as bacc
    import concourse.mybir as mybir
    from concourse.tile import TileContext
    from concourse.tile_rust import add_dep_helper

    f32 = mybir.dt.float32
    A = mybir.AluOpType
    AF = mybir.ActivationFunctionType

    nc = bacc.Bacc(None, target_bir_lowering=False)
    ctrl_d = nc.declare_dram_parameter("ctrl", [P, NPT * H * 3], f32, isOutput=False)
    x0_d = nc.declare_dram_parameter("x0p", [P, 12], f32, isOutput=False)
    scl_d = nc.declare_dram_parameter("sclp", [P, 16], f32, isOutput=False)
    out_d = nc.declare_dram_parameter("out", [P, NPT * SW], f32, isOutput=True)

    with TileContext(nc) as tc:
        v = nc.vector
        sc = nc.scalar
        gp = nc.gpsimd
        sy = nc.sync

        with tc.tile_pool(name="pers", bufs=1) as pp, \
             tc.tile_pool(name="ctrlp", bufs=2) as cp, \
             tc.tile_pool(name="wk1", bufs=1) as w1, \
             tc.tile_pool(name="wk2", bufs=2) as w2, \
             tc.tile_pool(name="stgp", bufs=3) as sp, \
             tc.tile_pool(name="psp", bufs=2, space="PSUM") as ps:

            x0s = pp.tile([P, 12], f32, tag="x0s")
            scl = pp.tile([P, 16], f32, tag="scl")
            steer = scl[:, 0:1]
            brake = scl[:, 1:2]
            thr = scl[:, 2:3]
            b65 = scl[:, 3:4]
            t28 = scl[:, 4:5]
            lo75 = scl[:, 5:6]
            invf981 = scl[:, 6:7]
            s0v = scl[:, 7:8]
            psi0 = scl[:, 8:9]
            px0 = scl[:, 9:10]
            py0 = scl[:, 10:11]
            vx010 = scl[:, 11:12]
            vy010 = scl[:, 12:13]
            hpiv = scl[:, 13:14]
            psi0dt = scl[:, 14:15]

            # scan masks (same pattern for every chunk)
            maskc = pp.tile([P, CF], f32, tag="maskc")
            v.memset(maskc[:], 1.0)
            mc3 = maskc[:].rearrange("p (n h) -> p n h", n=NG)
            v.memset(mc3[:, :, 0:1], 0.0)
            bigm = pp.tile([P, CF], f32, tag="bigm")
            v.memset(bigm[:], 0.0)
            bg3 = bigm[:].rearrange("p (n h) -> p n h", n=NG)
            v.memset(bg3[:, :, 0:1], BIG)

            # persistent control transforms (sg1/sg2 interleaved pairwise)
            f16 = mybir.dt.float16
            th = pp.tile([P, F], f16, tag="th")
            sg = pp.tile([P, 2 * F], f16, tag="sg")
            accDT = pp.tile([P, F], f32, tag="accDT")
            th3 = th[:].rearrange("p (n h) -> p n h", n=NPT)
            sg4 = sg[:].rearrange("p (n h c) -> p n h c", n=NPT, c=2)


            ctrls = {}

            def ctrl_dma(q):
                ctrl = cp.tile([P, QW], f32, tag="ctrlq")
                sy.dma_start(out=ctrl[:], in_=ctrl_d[:, q * QW:(q + 1) * QW])
                ctrls[q] = ctrl

            sig_grp = {}
            trig_grp = {}

            def phaseA_act(q):
                n0 = q * NG
                nsl = slice(n0, n0 + NG)
                c4 = ctrls.pop(q)[:].rearrange("p (n h c) -> p n h c", n=NG, h=H)
                i_sg = sc.activation(sg4[:, nsl, :, :], c4[:, :, :, 1:3],
                                     AF.Sigmoid)
                i_th = sc.activation(th3[:, nsl, :], c4[:, :, :, 0], AF.Tanh)
                sig_grp.setdefault(0, []).extend([i_sg, i_th])

            def phaseA_acc(q):
                n0 = q * NG
                nsl = slice(n0, n0 + NG)
                csl = slice(q * CF, (q + 1) * CF)
                fb65 = w1.tile([P, CF], f32, tag="pa_a")
                gp.tensor_scalar(fb65[:], sg4[:, nsl, :, 0], b65, None, A.mult)
                t3 = w1.tile([P, CF], f32, tag="pa_b")
                v.scalar_tensor_tensor(t3[:], sg4[:, nsl, :, 1], t28, fb65[:],
                                       A.mult, A.subtract)
                gp.tensor_scalar(accDT[:, csl], t3[:], 0.3, lo75, A.min, A.max)

            def phaseA(q):
                phaseA_act(q)
                phaseA_acc(q)

            stgs = {}
            carry = {}

            def stage1(g):
                n0 = g * NG
                nsl = slice(n0, n0 + NG)
                csl = slice(g * CF, (g + 1) * CF)

                stg = sp.tile([P, CHW], f32, tag="stg")
                stgs[g] = stg
                s4 = stg[:].rearrange("p (n t c) -> p n t c", n=NG, t=HP1)

                # t=0 slice: full x0 row broadcast over rollouts
                x0b = x0s[:, None, 0:12].broadcast_to([P, NG, 12])
                gp.tensor_scalar(s4[:, :, 0, :], x0b, 1.0, None, A.mult)

                # ch9 delta = steer*tanh(u0); dc = clip(delta, +-0.75)
                gp.tensor_scalar(s4[:, :, 1:, 9], th3[:, nsl, :], steer, None,
                                A.mult)
                dc = w1.tile([P, CF], f32, tag="dc")
                dc3 = dc[:].rearrange("p (n h) -> p n h", n=NG)
                gp.tensor_scalar(dc3, s4[:, :, 1:, 9], 0.75, -0.75, A.min, A.max)

                # trig of steering
                tg = trig_grp.setdefault(0, [])
                sin_d = w2.tile([P, CF], f32, tag="sin_d", bufs=3)
                tg.append(sc.activation(sin_d[:], dc[:], AF.Sin))
                cos_d = w2.tile([P, CF], f32, tag="cos_d")
                tg.append(sc.activation(cos_d[:], dc[:], AF.Sin, bias=hpiv))
                # ch10: brake*sg1
                gp.tensor_scalar(s4[:, :, 1:, 10], sg4[:, nsl, :, 0], brake,
                                 None, A.mult)

                # speed2 scans: C=cumsum(accDT); s2 = (C+s0) - runmin(min(C+s0,0))
                C = w1.tile([P, CF], f32, tag="C")
                v.tensor_tensor_scan(C[:], maskc[:], accDT[:, csl], 0.0,
                                     A.mult, A.add)
                Bm = w1.tile([P, CF], f32, tag="w_a")
                v.tensor_scalar(Bm[:], C[:], s0v, 0.0, A.add, A.min)
                mrun = w1.tile([P, CF], f32, tag="w_b")
                v.tensor_tensor_scan(mrun[:], bigm[:], Bm[:], 0.0, A.add, A.min)
                s2 = w2.tile([P, CF], f32, tag="s2", bufs=3)
                v.scalar_tensor_tensor(s2[:], C[:], s0v, mrun[:], A.add,
                                       A.subtract)

                rc = w1.tile([P, CF], f32, tag="rc")
                v.reciprocal_approx_fast(rc[:], cos_d[:])
                tan = w1.tile([P, CF], f32, tag="tan")
                gp.tensor_tensor(tan[:], sin_d[:], rc[:], A.mult)
                tan3 = tan[:].rearrange("p (n h) -> p n h", n=NG)
                tg.append(sc.activation(s4[:, :, 1:, 8], tan3, AF.Arctan,
                                        scale=0.45))
                mchi = w1.tile([P, CF], f32, tag="mchi")
                v.tensor_scalar(mchi[:], s2[:], 2.0, invf981, A.max, A.mult)
                imc = w1.tile([P, CF], f32, tag="imc")
                v.reciprocal_approx_fast(imc[:], mchi[:])

                # yaw_rate into ch5 (strided), psi cumsum reads it back strided
                rawc = w1.tile([P, CF], f32, tag="w_a")
                v.scalar_tensor_tensor(rawc[:], s2[:], 1.0 / WB, tan[:],
                                       A.mult, A.mult)
                clpc = w1.tile([P, CF], f32, tag="w_b")
                v.tensor_scalar(clpc[:], rawc[:], 1.0, -1.0, A.min, A.max)
                yawc = w1.tile([P, CF], f32, tag="yawc")
                v.scalar_tensor_tensor(yawc[:], imc[:], 0.15, clpc[:],
                                       A.max, A.mult)
                yw3 = yawc[:].rearrange("p (n h) -> p n h", n=NG)
                gp.tensor_scalar(s4[:, :, 1:, 5], yw3, 1.0, None, A.mult)
                # inject psi0/DT at each rollout start so the cumsum carries psi0
                v.tensor_scalar(yw3[:, :, 0], yw3[:, :, 0], psi0dt, None, A.add)
                Fp = w1.tile([P, CF], f32, tag="Fp")
                Fp3 = Fp[:].rearrange("p (n h) -> p n h", n=NG)
                v.tensor_tensor_scan(Fp[:], maskc[:], yawc[:], 0.0, A.mult, A.add)
                sc.activation(s4[:, :, 1:, 2], Fp3, AF.Identity, scale=DT)

                # arg = Fp*DT + beta in one DVE op; wrap for sin/cos
                argc = w2.tile([P, CF], f32, tag="argc")
                v.affine_then_add(argc[:].rearrange("p (n h) -> p n h", n=NG),
                                  Fp3, s4[:, :, 1:, 8], DT, 0.0)
                argw = ps.tile([P, CF], f32, tag="argw")
                v.add_range_wrap(argw[:], argc[:], 0.0, PI, 2 * PI)
                cwv = ps.tile([P, CF], f32, tag="cwv")
                v.add_range_wrap(cwv[:], argc[:], PI / 2, PI, 2 * PI)
                # ch11 late so it stays off the ACT critical path
                sc.activation(s4[:, :, 1:, 11], sg4[:, nsl, :, 1], AF.Identity,
                              scale=thr)
                carry[g] = (s2, argw, cwv)

            def stage2(g):
                n0 = g * NG
                nsl = slice(n0, n0 + NG)
                stg = stgs.pop(g)
                s4 = stg[:].rearrange("p (n t c) -> p n t c", n=NG, t=HP1)
                s2, argw, cwv = carry.pop(g)

                tg = trig_grp[0]
                sinA = w1.tile([P, CF], f32, tag="sinA")
                tg.append(sc.activation(sinA[:], argw[:], AF.Sin))
                cosA = w1.tile([P, CF], f32, tag="cosA")
                tg.append(sc.activation(cosA[:], cwv[:], AF.Sin))

                # velocities scaled by 10 (w = 10*s2*cos); ch3 = 0.1*w
                w10x = w1.tile([P, CF], f32, tag="w10x")
                v.scalar_tensor_tensor(w10x[:], s2[:], 10.0, cosA[:],
                                       A.mult, A.mult)
                w10y = w1.tile([P, CF], f32, tag="w10y")
                v.scalar_tensor_tensor(w10y[:], s2[:], 10.0, sinA[:],
                                       A.mult, A.mult)
                wx3 = w10x[:].rearrange("p (n h) -> p n h", n=NG)
                wy3 = w10y[:].rearrange("p (n h) -> p n h", n=NG)
                sc.activation(s4[:, :, 1:, 3], wx3, AF.Identity, scale=0.1)
                sc.activation(s4[:, :, 1:, 4], wy3, AF.Identity, scale=0.1)

                # px/py cumsums (of 10*v, so scale DT/10)
                Fx = w1.tile([P, CF], f32, tag="Fx")
                Fx3 = Fx[:].rearrange("p (n h) -> p n h", n=NG)
                v.tensor_tensor_scan(Fx[:], maskc[:], w10x[:], 0.0,
                                     A.mult, A.add)
                sc.activation(s4[:, :, 1:, 0], Fx3, AF.Identity, bias=px0,
                              scale=DT / 10.0)
                Fy = w1.tile([P, CF], f32, tag="Fy")
                Fy3 = Fy[:].rearrange("p (n h) -> p n h", n=NG)
                v.tensor_tensor_scan(Fy[:], maskc[:], w10y[:], 0.0,
                                     A.mult, A.add)
                sc.activation(s4[:, :, 1:, 1], Fy3, AF.Identity, bias=py0,
                              scale=DT / 10.0)

                # ax/ay: diffs of w10 are already scaled
                gp.tensor_tensor(s4[:, :, 2:, 6], wx3[:, :, 1:],
                                 wx3[:, :, :H - 1], A.subtract)
                v.tensor_scalar(s4[:, :, 1, 6], wx3[:, :, 0], 1.0, vx010,
                                A.mult, A.subtract)
                gp.tensor_tensor(s4[:, :, 2:, 7], wy3[:, :, 1:],
                                 wy3[:, :, :H - 1], A.subtract)
                v.tensor_scalar(s4[:, :, 1, 7], wy3[:, :, 0], 1.0, vy010,
                                A.mult, A.subtract)

                sy.dma_start(out=out_d[:, g * CHW:(g + 1) * CHW], in_=stg[:])

            # emission: ALL input DMAs first so nothing queues behind outputs;
            # ctrl q0 leads so phase A starts as early as possible
            ctrl_dma(0)
            sy.dma_start(out=x0s[:], in_=x0_d[:])
            sy.dma_start(out=scl[:], in_=scl_d[:])
            for q in range(1, NCH):
                ctrl_dma(q)
            phaseA(0)
            phaseA(1)
            phaseA(2)
            phaseA(3)
            stage1(0)
            stage2(0)
            stage1(1)
            stage2(1)
            stage1(2)
            stage2(2)
            stage1(3)
            stage2(3)

            # activation-table grouping: all sigmoid/tanh | all trig
            groups = [sig_grp[0], trig_grp[0]]
            for ga, gb in zip(groups, groups[1:]):
                for ib in gb:
                    for ia in ga:
                        add_dep_helper(ib.ins, ia.ins, reason="act table order")

    nc.compile()
    return nc


def _get_built():
    global _BUILT
    if _BUILT is None:
        _BUILT = _build_kernel()
    return _BUILT


def _make_scalars(x0, deg):
    """Per-batch scalar table [B, 16] (host precompute of deg-derived scales)."""
    steer = np.maximum(deg[:, 0], 0.05)
    brake = np.maximum(deg[:, 1], 0.05)
    thr = np.maximum(deg[:, 2], 0.05)
    fric = np.maximum(deg[:, 4], 0.1)
    vx0 = x0[:, 3]
    vy0 = x0[:, 4]
    scl = np.zeros((B, 16), np.float32)
    scl[:, 0] = steer
    scl[:, 1] = brake
    scl[:, 2] = thr
    scl[:, 3] = 0.65 * brake
    scl[:, 4] = 0.28 * thr
    scl[:, 5] = -0.75 * fric
    scl[:, 6] = 1.0 / (9.81 * fric)
    scl[:, 7] = np.sqrt(vx0 * vx0 + vy0 * vy0 + 1e-6)
    scl[:, 8] = x0[:, 2]
    scl[:, 9] = x0[:, 0]
    scl[:, 10] = x0[:, 1]
    scl[:, 11] = 10.0 * vx0
    scl[:, 12] = 10.0 * vy0
    scl[:, 13] = PI / 2
    scl[:, 14] = 10.0 * x0[:, 2]
    return scl


def _run(x0, controls, deg, trace=False):
    from concourse.bass_utils import run_bass_kernel_spmd

    x0 = np.ascontiguousarray(x0, dtype=np.float32)
    controls = np.ascontiguousarray(controls, dtype=np.float32)
    deg = np.ascontiguousarray(deg, dtype=np.float32)
    scl = _make_scalars(x0, deg)

    nc = _get_built()
    rep = P // BC
    in_maps = []
    for c in range(NCORES):
        sl = slice(c * BC, (c + 1) * BC)
        ctrl_c = controls[sl].reshape(R, H * 3).reshape(P, NPT * H * 3)
        in_maps.append({
            "ctrl": np.ascontiguousarray(ctrl_c),
            "x0p": np.ascontiguousarray(np.repeat(x0[sl], rep, axis=0)),
            "sclp": np.ascontiguousarray(np.repeat(scl[sl], rep, axis=0)),
        })

    res = run_bass_kernel_spmd(nc, in_maps, list(range(NCORES)), trace=trace)
    outs = []
    for c in range(NCORES):
        o = np.asarray(res.results[c]["out"])
        outs.append(o.reshape(R, HP1, CW).reshape(BC, L, HP1, CW))
    return np.concatenate(outs, axis=0), res


def kernel(x0: np.ndarray, controls: np.ndarray, deg: np.ndarray) -> np.ndarray:
    out, _ = _run(x0, controls, deg)
    return out


if __name__ == "__main__":
    rng = np.random.default_rng(0)
    x0 = rng.standard_normal((B, 12)).astype(np.float32)
    controls = rng.standard_normal((B, L, H, 3)).astype(np.float32)
    deg = rng.random((B, 5)).astype(np.float32)
    out = kernel(x0, controls, deg)
    print("out", out.shape, out.dtype)
